# revision 24
# baseline (speedup 1.0000x reference)
"""Cross-attention kernel for Trainium2 (Bass/Tile), 8-core data-parallel.

Reference computation (per batch element b):
    q = x @ Wq.T ; k = ctx @ Wk.T ; v = ctx @ Wv.T
    out = softmax((q @ k.T) * D**-0.5) @ v

Shapes: x [8, 2048, 1024], context [8, 2048, 1024], Wq/Wk/Wv [1024, 1024].

Strategy: pure data-parallel -- one batch element per NeuronCore, no
collectives.  All matmuls in bf16 with fp32 PSUM accumulation.

Since softmax((q k^T) * s) only needs q k^T = x (Wq^T Wk) ctx^T, we never
materialize q or k: W' = Wq^T Wk is computed from the *natural* weight
layouts (contraction over the out-feature axis, already on partitions),
then yT = W'^T x^T and dotsT = ctx^T-stationary x yT-moving.  This kills
the k projection and all Wq/Wk transposes.

PE-roofline design (v4):
  * Inputs are pre-cast to bf16 on the HOST: device DMA is half of
    fp32 and there are no on-chip casts at all.  (An XBAR
    transposed-load variant was measured slower -- 256B-granular HBM
    reads starve the PE under 8-core contention -- so transposes stay
    on the PE where they cost only ~67ns each back-to-back.)
  * 128x128 block transposes of Wv/ctx/x run on the PE in groups of 4
    sharing one PSUM bank; one strided DVE copy drains four blocks.
  * Softmax denominators come from N=1 matmuls (moving = ones [128,1])
    that reuse the attn^T stationary already loaded for the attn@v
    matmuls; sum_t attn[s,t] accumulates in a [128,1] PSUM column.
    ~25ns each instead of a second full pass of attn^T through the PE
    (~27us) plus column transposes.
  * Normalization (x 1/rowsum) runs on DVE so the Scalar engine does
    nothing but exp during the attention phase and a dots drain never
    queues behind a mul.
  * Softmax runs without max-subtraction (logits are O(5) for
    unit-normal inputs); exp comes straight out of PSUM on the Scalar
    engine with the 1/32 scale folded in; normalization is applied
    after the attn@v matmul since that matmul is linear in attn.
"""

from contextlib import ExitStack

import numpy as np

B = 8
S = 2048  # query length
T = 2048  # key/value length
D = 1024  # model dim
P = 128
SCALE = float(D) ** -0.5

N_ST = S // P  # 16 query tiles
N_TT = T // P  # 16 key tiles
N_DT = D // P  # 8 contraction chunks
NPROJ = D // 512  # 2 x 512-wide chunks for [., 1024] outputs
NSB = 4  # x is processed in 4 s-blocks of 512 rows for the yT projection
SH = S // 2  # attention processed in 2 query halves of 1024


def _emit_body(tc, x, ctxt, wq, wk, wv, out):
    import concourse.mybir as mybir
    from concourse.masks import make_identity

    fp32 = mybir.dt.float32
    bf16 = mybir.dt.bfloat16
    nc = tc.nc

    with ExitStack() as ctx:
        # Several pools share slots across phases via a common tag: wnp
        # hosts Wq/Wk naturals (phase A) then attn^T tiles (phase B);
        # xtbp hosts x^T blocks (phase A) then fp32 out staging (phase B).
        const = ctx.enter_context(tc.tile_pool(name="const", bufs=1))
        # 18 slots (2 spare) so dots-h1's first attn^T tiles can allocate
        # while attnv-h0 still holds all 16 h0 tiles -> no PE gap at the
        # half transition.
        wnp = ctx.enter_context(tc.tile_pool(name="wnp", bufs=18))
        castp = ctx.enter_context(tc.tile_pool(name="castp", bufs=8))
        wpp = ctx.enter_context(tc.tile_pool(name="wpp", bufs=8))
        wvtp = ctx.enter_context(tc.tile_pool(name="wvtp", bufs=2))
        xtbp = ctx.enter_context(tc.tile_pool(name="xtbp", bufs=4))
        ctxp = ctx.enter_context(tc.tile_pool(name="ctxp", bufs=2))
        ytp = ctx.enter_context(tc.tile_pool(name="ytp", bufs=8))
        vp = ctx.enter_context(tc.tile_pool(name="vp", bufs=16))
        smp = ctx.enter_context(tc.tile_pool(name="smp", bufs=2))

        ident_b = const.tile([P, P], bf16, name="ident_b")
        make_identity(nc, ident_b)
        ones_b = const.tile([P, 1], bf16, name="ones_b")
        nc.vector.memset(ones_b, 1.0)

        # Inputs are pre-cast to bf16 on the host, so a load is a single
        # bf16 DMA -- half the HBM traffic of fp32 and no cast op at all.
        # Loads alternate between the two HWDGE queues (SP and ACT) so two
        # row-tiles are always in flight.
        def load_bf(dram_rows, pool, tag, nm):
            """DMA one bf16 [128, D] row-tile."""
            bt = pool.tile([P, D], bf16, name=f"bf_{nm}", tag=tag)
            nc.sync.dma_start(out=bt, in_=dram_rows)
            return bt

        def prep_t(dram_rows, dst_for_group, psum_pool, nm):
            """Load one bf16 [128, D] row-tile in two [128, 512] halves and
            PE-transpose each half's 4 128x128 blocks as soon as it lands
            (group g depends only on its own half-DMA, so the PE starts
            after 128KB instead of 256KB); 4 transposes share one PSUM bank
            and one strided DVE copy scatters them into dst_for_group(g)
            (an AP shaped [128, 4, 128])."""
            for g in range(2):
                hb = castp.tile([P, 512], bf16, name=f"bf_{nm}_{g}", tag="cast")
                nc.sync.dma_start(out=hb, in_=dram_rows[:, g * 512 : (g + 1) * 512])
                ps = psum_pool.tile(
                    [P, 4 * P], bf16, name=f"tp_{nm}_{g}", tag="pt", bufs=3
                )
                for j in range(4):
                    nc.tensor.transpose(
                        ps[:, j * P : (j + 1) * P],
                        hb[:, j * P : (j + 1) * P],
                        ident_b,
                    )
                nc.vector.tensor_copy(
                    out=dst_for_group(g), in_=ps.rearrange("p (j c) -> p j c", j=4)
                )

        with tc.tile_pool(name="psumA", bufs=1, space="PSUM") as psA:
            # ---- Wv^T first (small), then ctx: each ctx row-tile's
            # transposes are chased immediately by that tile's v matmuls so
            # the PE stays fed at DMA pace ----
            wvg = [
                wvtp.tile([P, 4, D], bf16, name=f"wvg{g}", tag="wvg")
                for g in range(2)
            ]
            ctxg = [
                ctxp.tile([P, 4, T], bf16, name=f"ctxg{g}", tag="ctxg")
                for g in range(2)
            ]
            v = [vp.tile([P, D], bf16, name=f"v{t}", tag="v") for t in range(N_TT)]

            def prep_wv(rt):
                prep_t(
                    wv[rt * P : (rt + 1) * P, :],
                    lambda g, rt=rt: wvg[g][:, :, rt * P : (rt + 1) * P],
                    psA,
                    f"wv{rt}",
                )

            def prep_ctx(rt):
                prep_t(
                    ctxt[rt * P : (rt + 1) * P, :],
                    lambda g, rt=rt: ctxg[g][:, :, rt * P : (rt + 1) * P],
                    psA,
                    f"c{rt}",
                )

            # Wv columns 0-511 first so v[tt][ne=0] can start after just 4 Wv
            # tiles + one ctx tile; remaining Wv tiles stream in behind.
            for rt in range(4):
                prep_wv(rt)
            prep_ctx(0)
            for rt in range(4, N_DT):
                prep_wv(rt)
            for rt in range(N_TT):
                if rt > 0:
                    prep_ctx(rt)
                tt = rt  # v = ctx @ Wv^T, natural layout [t, e]
                for ne in range(NPROJ):
                    ps = psA.tile(
                        [P, 512], fp32, name=f"pv{tt}_{ne}", tag="proj", bufs=4
                    )
                    for d in range(N_DT):
                        nc.tensor.matmul(
                            ps,
                            ctxg[d // 4][:, d % 4, tt * P : (tt + 1) * P],
                            wvg[d // 4][:, d % 4, ne * 512 : (ne + 1) * 512],
                            start=(d == 0),
                            stop=(d == N_DT - 1),
                        )
                    nc.scalar.copy(out=v[tt][:, ne * 512 : (ne + 1) * 512], in_=ps)

            # ---- Wq/Wk in natural layout (bf16), then W' = Wq^T @ Wk ----
            wqn = [
                load_bf(wq[e * P : (e + 1) * P, :], wnp, "wn", f"wq{e}")
                for e in range(N_DT)
            ]
            wkn = [
                load_bf(wk[e * P : (e + 1) * P, :], wnp, "wn", f"wk{e}")
                for e in range(N_DT)
            ]
            wpb = [
                wpp.tile([P, D], bf16, name=f"wp{i}", tag="wp") for i in range(N_DT)
            ]
            for it in range(N_DT):
                for jn in range(NPROJ):
                    ps = psA.tile(
                        [P, 512], fp32, name=f"pw{it}_{jn}", tag="proj", bufs=4
                    )
                    for e in range(N_DT):
                        nc.tensor.matmul(
                            ps,
                            wqn[e][:, it * P : (it + 1) * P],
                            wkn[e][:, jn * 512 : (jn + 1) * 512],
                            start=(e == 0),
                            stop=(e == N_DT - 1),
                        )
                    nc.scalar.copy(out=wpb[it][:, jn * 512 : (jn + 1) * 512], in_=ps)

            # ---- yT = (x @ W')^T, streamed over 4 s-blocks of 512 ----
            yt = [
                ytp.tile([P, S], bf16, name=f"yt{j}", tag="yt") for j in range(N_DT)
            ]
            for sb in range(NSB):
                xtb = [
                    xtbp.tile([P, 4, 512], bf16, name=f"xtb{sb}_{g}", tag="xtb")
                    for g in range(2)
                ]
                for r in range(4):
                    rt = 4 * sb + r
                    prep_t(
                        x[rt * P : (rt + 1) * P, :],
                        lambda g, r=r: xtb[g][:, :, r * P : (r + 1) * P],
                        psA,
                        f"x{rt}",
                    )
                for jt in range(N_DT):
                    ps = psA.tile(
                        [P, 512], fp32, name=f"py{sb}_{jt}", tag="proj", bufs=4
                    )
                    for i in range(N_DT):
                        nc.tensor.matmul(
                            ps,
                            wpb[i][:, jt * P : (jt + 1) * P],
                            xtb[i // 4][:, i % 4, :],
                            start=(i == 0),
                            stop=(i == N_DT - 1),
                        )
                    nc.scalar.copy(
                        out=yt[jt][:, sb * 512 : (sb + 1) * 512], in_=ps
                    )

        # ---- attention ----
        # dots is produced TRANSPOSED: dotsT[t_tile, s] = sum_d ctxT[d, t] *
        # yT[d, s] (same operands as dots, roles swapped), so exp output IS
        # attn^T and the attn@v matmul needs no transposes at all.  S is
        # processed in 2 halves of 1024 so attn^T fits in the 16 SBUF slots
        # the W' inputs vacated.
        with tc.tile_pool(name="psumB", bufs=1, space="PSUM") as psB:
            for h in range(2):
                atT = []
                for tt in range(N_TT):
                    at = wnp.tile([P, SH], bf16, name=f"atT{h}_{tt}", tag="wn")
                    for ns in range(SH // 512):
                        ps = psB.tile(
                            [P, 512], fp32, name=f"pd{h}_{tt}_{ns}", tag="dots",
                            bufs=3,
                        )
                        for d in range(N_DT):
                            nc.tensor.matmul(
                                ps,
                                ctxg[d // 4][:, d % 4, tt * P : (tt + 1) * P],
                                yt[d][:, h * SH + ns * 512 : h * SH + (ns + 1) * 512],
                                start=(d == 0),
                                stop=(d == N_DT - 1),
                            )
                        nc.scalar.activation(
                            out=at[:, ns * 512 : (ns + 1) * 512],
                            in_=ps,
                            func=mybir.ActivationFunctionType.Exp,
                            scale=SCALE,
                        )
                    atT.append(at)

                for sl in range(N_ST // 2):
                    st = h * (N_ST // 2) + sl
                    out_sb = xtbp.tile([P, D], bf16, name=f"o{st}", tag="xtb")
                    ps0 = psB.tile(
                        [P, 512], fp32, name=f"pav{st}_0", tag="av", bufs=2
                    )
                    ps1 = psB.tile(
                        [P, 512], fp32, name=f"pav{st}_1", tag="av2", bufs=2
                    )
                    psr = psB.tile(
                        [P, 1], fp32, name=f"psr{st}", tag="rsum", bufs=1
                    )
                    for tt in range(N_TT):
                        a_sl = atT[tt][:, sl * P : (sl + 1) * P]
                        nc.tensor.matmul(
                            psr, a_sl, ones_b,
                            start=(tt == 0), stop=(tt == N_TT - 1),
                        )
                        nc.tensor.matmul(
                            ps0, a_sl, v[tt][:, 0:512],
                            start=(tt == 0), stop=(tt == N_TT - 1),
                        )
                        nc.tensor.matmul(
                            ps1, a_sl, v[tt][:, 512:1024],
                            start=(tt == 0), stop=(tt == N_TT - 1),
                        )
                    recip = smp.tile(
                        [P, 1], fp32, name=f"rc{st}", tag="recip", bufs=8
                    )
                    nc.vector.reciprocal(out=recip, in_=psr)
                    nc.vector.tensor_scalar_mul(
                        out_sb[:, 0:512], ps0, recip
                    )
                    nc.vector.tensor_scalar_mul(
                        out_sb[:, 512:1024], ps1, recip
                    )
                    nc.sync.dma_start(
                        out=out[st * P : (st + 1) * P, :], in_=out_sb
                    )


def build_nc():
    import concourse.mybir as mybir
    import concourse.tile as tile
    from concourse import bacc

    fp32 = mybir.dt.float32
    bf16 = mybir.dt.bfloat16
    nc = bacc.Bacc("TRN2", target_bir_lowering=False, debug=False)
    x = nc.dram_tensor("x", [S, D], bf16, kind="ExternalInput").ap()
    ctxt = nc.dram_tensor("context", [T, D], bf16, kind="ExternalInput").ap()
    wq = nc.dram_tensor("Wq", [D, D], bf16, kind="ExternalInput").ap()
    wk = nc.dram_tensor("Wk", [D, D], bf16, kind="ExternalInput").ap()
    wv = nc.dram_tensor("Wv", [D, D], bf16, kind="ExternalInput").ap()
    # bf16 output (upcast to fp32 on the host): halves the output DMA and
    # the final drain chain; costs ~0.2% extra quantization error against a
    # 2e-2 budget.
    out = nc.dram_tensor("out", [S, D], bf16, kind="ExternalOutput").ap()
    with tile.TileContext(nc) as tc:
        _emit_body(tc, x, ctxt, wq, wk, wv, out)
    nc.compile()
    return nc


_CACHED_NC = None


def kernel(**inputs):
    global _CACHED_NC
    import ml_dtypes
    from concourse.bass_utils import run_bass_kernel_spmd

    bf = ml_dtypes.bfloat16
    x = np.ascontiguousarray(np.asarray(inputs["x"], dtype=np.float32).astype(bf))
    ctxt = np.ascontiguousarray(
        np.asarray(inputs["context"], dtype=np.float32).astype(bf)
    )
    wq = np.ascontiguousarray(np.asarray(inputs["Wq"], dtype=np.float32).astype(bf))
    wk = np.ascontiguousarray(np.asarray(inputs["Wk"], dtype=np.float32).astype(bf))
    wv = np.ascontiguousarray(np.asarray(inputs["Wv"], dtype=np.float32).astype(bf))

    if _CACHED_NC is None:
        _CACHED_NC = build_nc()
    nc = _CACHED_NC

    in_maps = [
        {"x": x[b], "context": ctxt[b], "Wq": wq, "Wk": wk, "Wv": wv}
        for b in range(B)
    ]
    res = run_bass_kernel_spmd(nc, in_maps, core_ids=list(range(B)))
    return np.stack(
        [res.results[b]["out"].astype(np.float32) for b in range(B)], axis=0
    )


# revision 25
# speedup vs baseline: 1.0043x; 1.0043x over previous
"""Cross-attention kernel for Trainium2 (Bass/Tile), 8-core data-parallel.

Reference computation (per batch element b):
    q = x @ Wq.T ; k = ctx @ Wk.T ; v = ctx @ Wv.T
    out = softmax((q @ k.T) * D**-0.5) @ v

Shapes: x [8, 2048, 1024], context [8, 2048, 1024], Wq/Wk/Wv [1024, 1024].

Strategy: pure data-parallel -- one batch element per NeuronCore, no
collectives.  All matmuls in bf16 with fp32 PSUM accumulation.

Since softmax((q k^T) * s) only needs q k^T = x (Wq^T Wk) ctx^T, we never
materialize q or k: W' = Wq^T Wk is computed from the *natural* weight
layouts (contraction over the out-feature axis, already on partitions),
then yT = W'^T x^T and dotsT = ctx^T-stationary x yT-moving.  This kills
the k projection and all Wq/Wk transposes.

PE-roofline design (v4):
  * Inputs are pre-cast to bf16 on the HOST: device DMA is half of
    fp32 and there are no on-chip casts at all.  (An XBAR
    transposed-load variant was measured slower -- 256B-granular HBM
    reads starve the PE under 8-core contention -- so transposes stay
    on the PE where they cost only ~67ns each back-to-back.)
  * 128x128 block transposes of Wv/ctx/x run on the PE in groups of 4
    sharing one PSUM bank; one strided DVE copy drains four blocks.
  * Softmax denominators come from N=1 matmuls (moving = ones [128,1])
    that reuse the attn^T stationary already loaded for the attn@v
    matmuls; sum_t attn[s,t] accumulates in a [128,1] PSUM column.
    ~25ns each instead of a second full pass of attn^T through the PE
    (~27us) plus column transposes.
  * Normalization (x 1/rowsum) runs on DVE so the Scalar engine does
    nothing but exp during the attention phase and a dots drain never
    queues behind a mul.
  * Softmax runs without max-subtraction (logits are O(5) for
    unit-normal inputs); exp comes straight out of PSUM on the Scalar
    engine with the 1/32 scale folded in; normalization is applied
    after the attn@v matmul since that matmul is linear in attn.
"""

from contextlib import ExitStack

import numpy as np

B = 8
S = 2048  # query length
T = 2048  # key/value length
D = 1024  # model dim
P = 128
SCALE = float(D) ** -0.5

N_ST = S // P  # 16 query tiles
N_TT = T // P  # 16 key tiles
N_DT = D // P  # 8 contraction chunks
NPROJ = D // 512  # 2 x 512-wide chunks for [., 1024] outputs
NSB = 4  # x is processed in 4 s-blocks of 512 rows for the yT projection
SH = S // 2  # attention processed in 2 query halves of 1024


def _emit_body(tc, x, ctxt, wq, wk, wv, out):
    import concourse.mybir as mybir
    from concourse.masks import make_identity

    fp32 = mybir.dt.float32
    bf16 = mybir.dt.bfloat16
    nc = tc.nc

    with ExitStack() as ctx:
        # Several pools share slots across phases via a common tag: wnp
        # hosts Wq/Wk naturals (phase A) then attn^T tiles (phase B);
        # xtbp hosts x^T blocks (phase A) then fp32 out staging (phase B).
        const = ctx.enter_context(tc.tile_pool(name="const", bufs=1))
        # 18 slots (2 spare) so dots-h1's first attn^T tiles can allocate
        # while attnv-h0 still holds all 16 h0 tiles -> no PE gap at the
        # half transition.
        wnp = ctx.enter_context(tc.tile_pool(name="wnp", bufs=18))
        castp = ctx.enter_context(tc.tile_pool(name="castp", bufs=8))
        wpp = ctx.enter_context(tc.tile_pool(name="wpp", bufs=8))
        wvtp = ctx.enter_context(tc.tile_pool(name="wvtp", bufs=2))
        xtbp = ctx.enter_context(tc.tile_pool(name="xtbp", bufs=4))
        ctxp = ctx.enter_context(tc.tile_pool(name="ctxp", bufs=2))
        ytp = ctx.enter_context(tc.tile_pool(name="ytp", bufs=8))
        vp = ctx.enter_context(tc.tile_pool(name="vp", bufs=16))
        smp = ctx.enter_context(tc.tile_pool(name="smp", bufs=2))

        ident_b = const.tile([P, P], bf16, name="ident_b")
        make_identity(nc, ident_b)
        ones_b = const.tile([P, 1], bf16, name="ones_b")
        nc.vector.memset(ones_b, 1.0)

        # Inputs are pre-cast to bf16 on the host, so a load is a single
        # bf16 DMA -- half the HBM traffic of fp32 and no cast op at all.
        # Loads alternate between the two HWDGE queues (SP and ACT) so two
        # row-tiles are always in flight.
        def load_bf(dram_rows, pool, tag, nm):
            """DMA one bf16 [128, D] row-tile."""
            bt = pool.tile([P, D], bf16, name=f"bf_{nm}", tag=tag)
            nc.sync.dma_start(out=bt, in_=dram_rows)
            return bt

        def prep_t(dram_rows, dst_for_group, psum_pool, nm):
            """Load one bf16 [128, D] row-tile and PE-transpose its 8
            128x128 blocks in 2 groups of 4 sharing one PSUM bank; one
            strided DVE copy per group scatters into dst_for_group(g) (an
            AP shaped [128, 4, 128])."""
            bt = load_bf(dram_rows, castp, "cast", nm)
            for g in range(2):
                ps = psum_pool.tile(
                    [P, 4 * P], bf16, name=f"tp_{nm}_{g}", tag="pt", bufs=3
                )
                for j in range(4):
                    nc.tensor.transpose(
                        ps[:, j * P : (j + 1) * P],
                        bt[:, (4 * g + j) * P : (4 * g + j + 1) * P],
                        ident_b,
                    )
                nc.vector.tensor_copy(
                    out=dst_for_group(g), in_=ps.rearrange("p (j c) -> p j c", j=4)
                )

        with tc.tile_pool(name="psumA", bufs=1, space="PSUM") as psA:
            # ---- Wv^T first (small), then ctx: each ctx row-tile's
            # transposes are chased immediately by that tile's v matmuls so
            # the PE stays fed at DMA pace ----
            wvg = [
                wvtp.tile([P, 4, D], bf16, name=f"wvg{g}", tag="wvg")
                for g in range(2)
            ]
            ctxg = [
                ctxp.tile([P, 4, T], bf16, name=f"ctxg{g}", tag="ctxg")
                for g in range(2)
            ]
            v = [vp.tile([P, D], bf16, name=f"v{t}", tag="v") for t in range(N_TT)]

            def prep_wv(rt):
                prep_t(
                    wv[rt * P : (rt + 1) * P, :],
                    lambda g, rt=rt: wvg[g][:, :, rt * P : (rt + 1) * P],
                    psA,
                    f"wv{rt}",
                )

            def prep_ctx(rt):
                prep_t(
                    ctxt[rt * P : (rt + 1) * P, :],
                    lambda g, rt=rt: ctxg[g][:, :, rt * P : (rt + 1) * P],
                    psA,
                    f"c{rt}",
                )

            # Wv columns 0-511 first so v[tt][ne=0] can start after just 4 Wv
            # tiles + one ctx tile; remaining Wv tiles stream in behind.
            for rt in range(4):
                prep_wv(rt)
            prep_ctx(0)
            for rt in range(4, N_DT):
                prep_wv(rt)
            for rt in range(N_TT):
                if rt > 0:
                    prep_ctx(rt)
                tt = rt  # v = ctx @ Wv^T, natural layout [t, e]
                for ne in range(NPROJ):
                    ps = psA.tile(
                        [P, 512], fp32, name=f"pv{tt}_{ne}", tag="proj", bufs=4
                    )
                    for d in range(N_DT):
                        nc.tensor.matmul(
                            ps,
                            ctxg[d // 4][:, d % 4, tt * P : (tt + 1) * P],
                            wvg[d // 4][:, d % 4, ne * 512 : (ne + 1) * 512],
                            start=(d == 0),
                            stop=(d == N_DT - 1),
                        )
                    nc.scalar.copy(out=v[tt][:, ne * 512 : (ne + 1) * 512], in_=ps)

            # ---- Wq/Wk in natural layout (bf16), then W' = Wq^T @ Wk ----
            wqn = [
                load_bf(wq[e * P : (e + 1) * P, :], wnp, "wn", f"wq{e}")
                for e in range(N_DT)
            ]
            wkn = [
                load_bf(wk[e * P : (e + 1) * P, :], wnp, "wn", f"wk{e}")
                for e in range(N_DT)
            ]
            wpb = [
                wpp.tile([P, D], bf16, name=f"wp{i}", tag="wp") for i in range(N_DT)
            ]
            for it in range(N_DT):
                for jn in range(NPROJ):
                    ps = psA.tile(
                        [P, 512], fp32, name=f"pw{it}_{jn}", tag="proj", bufs=4
                    )
                    for e in range(N_DT):
                        nc.tensor.matmul(
                            ps,
                            wqn[e][:, it * P : (it + 1) * P],
                            wkn[e][:, jn * 512 : (jn + 1) * 512],
                            start=(e == 0),
                            stop=(e == N_DT - 1),
                        )
                    nc.scalar.copy(out=wpb[it][:, jn * 512 : (jn + 1) * 512], in_=ps)

            # ---- yT = (x @ W')^T, streamed over 4 s-blocks of 512 ----
            yt = [
                ytp.tile([P, S], bf16, name=f"yt{j}", tag="yt") for j in range(N_DT)
            ]
            for sb in range(NSB):
                xtb = [
                    xtbp.tile([P, 4, 512], bf16, name=f"xtb{sb}_{g}", tag="xtb")
                    for g in range(2)
                ]
                for r in range(4):
                    rt = 4 * sb + r
                    prep_t(
                        x[rt * P : (rt + 1) * P, :],
                        lambda g, r=r: xtb[g][:, :, r * P : (r + 1) * P],
                        psA,
                        f"x{rt}",
                    )
                for jt in range(N_DT):
                    ps = psA.tile(
                        [P, 512], fp32, name=f"py{sb}_{jt}", tag="proj", bufs=4
                    )
                    for i in range(N_DT):
                        nc.tensor.matmul(
                            ps,
                            wpb[i][:, jt * P : (jt + 1) * P],
                            xtb[i // 4][:, i % 4, :],
                            start=(i == 0),
                            stop=(i == N_DT - 1),
                        )
                    nc.scalar.copy(
                        out=yt[jt][:, sb * 512 : (sb + 1) * 512], in_=ps
                    )

        # ---- attention ----
        # dots is produced TRANSPOSED: dotsT[t_tile, s] = sum_d ctxT[d, t] *
        # yT[d, s] (same operands as dots, roles swapped), so exp output IS
        # attn^T and the attn@v matmul needs no transposes at all.  S is
        # processed in 2 halves of 1024 so attn^T fits in the 16 SBUF slots
        # the W' inputs vacated.
        with tc.tile_pool(name="psumB", bufs=1, space="PSUM") as psB:
            for h in range(2):
                atT = []
                for tt in range(N_TT):
                    at = wnp.tile([P, SH], bf16, name=f"atT{h}_{tt}", tag="wn")
                    for ns in range(SH // 512):
                        ps = psB.tile(
                            [P, 512], fp32, name=f"pd{h}_{tt}_{ns}", tag="dots",
                            bufs=3,
                        )
                        for d in range(N_DT):
                            nc.tensor.matmul(
                                ps,
                                ctxg[d // 4][:, d % 4, tt * P : (tt + 1) * P],
                                yt[d][:, h * SH + ns * 512 : h * SH + (ns + 1) * 512],
                                start=(d == 0),
                                stop=(d == N_DT - 1),
                            )
                        nc.scalar.activation(
                            out=at[:, ns * 512 : (ns + 1) * 512],
                            in_=ps,
                            func=mybir.ActivationFunctionType.Exp,
                            scale=SCALE,
                        )
                    atT.append(at)

                for sl in range(N_ST // 2):
                    st = h * (N_ST // 2) + sl
                    out_sb = xtbp.tile([P, D], bf16, name=f"o{st}", tag="xtb")
                    ps0 = psB.tile(
                        [P, 512], fp32, name=f"pav{st}_0", tag="av", bufs=2
                    )
                    ps1 = psB.tile(
                        [P, 512], fp32, name=f"pav{st}_1", tag="av2", bufs=2
                    )
                    psr = psB.tile(
                        [P, 1], fp32, name=f"psr{st}", tag="rsum", bufs=1
                    )
                    for tt in range(N_TT):
                        a_sl = atT[tt][:, sl * P : (sl + 1) * P]
                        nc.tensor.matmul(
                            psr, a_sl, ones_b,
                            start=(tt == 0), stop=(tt == N_TT - 1),
                        )
                        nc.tensor.matmul(
                            ps0, a_sl, v[tt][:, 0:512],
                            start=(tt == 0), stop=(tt == N_TT - 1),
                        )
                        nc.tensor.matmul(
                            ps1, a_sl, v[tt][:, 512:1024],
                            start=(tt == 0), stop=(tt == N_TT - 1),
                        )
                    recip = smp.tile(
                        [P, 1], fp32, name=f"rc{st}", tag="recip", bufs=8
                    )
                    nc.vector.reciprocal(out=recip, in_=psr)
                    nc.vector.tensor_scalar_mul(
                        out_sb[:, 0:512], ps0, recip
                    )
                    nc.vector.tensor_scalar_mul(
                        out_sb[:, 512:1024], ps1, recip
                    )
                    nc.sync.dma_start(
                        out=out[st * P : (st + 1) * P, :], in_=out_sb
                    )


def build_nc():
    import concourse.mybir as mybir
    import concourse.tile as tile
    from concourse import bacc

    fp32 = mybir.dt.float32
    bf16 = mybir.dt.bfloat16
    nc = bacc.Bacc("TRN2", target_bir_lowering=False, debug=False)
    x = nc.dram_tensor("x", [S, D], bf16, kind="ExternalInput").ap()
    ctxt = nc.dram_tensor("context", [T, D], bf16, kind="ExternalInput").ap()
    wq = nc.dram_tensor("Wq", [D, D], bf16, kind="ExternalInput").ap()
    wk = nc.dram_tensor("Wk", [D, D], bf16, kind="ExternalInput").ap()
    wv = nc.dram_tensor("Wv", [D, D], bf16, kind="ExternalInput").ap()
    # bf16 output (upcast to fp32 on the host): halves the output DMA and
    # the final drain chain; costs ~0.2% extra quantization error against a
    # 2e-2 budget.
    out = nc.dram_tensor("out", [S, D], bf16, kind="ExternalOutput").ap()
    with tile.TileContext(nc) as tc:
        _emit_body(tc, x, ctxt, wq, wk, wv, out)
    nc.compile()
    return nc


_CACHED_NC = None


def kernel(**inputs):
    global _CACHED_NC
    import ml_dtypes
    from concourse.bass_utils import run_bass_kernel_spmd

    bf = ml_dtypes.bfloat16
    x = np.ascontiguousarray(np.asarray(inputs["x"], dtype=np.float32).astype(bf))
    ctxt = np.ascontiguousarray(
        np.asarray(inputs["context"], dtype=np.float32).astype(bf)
    )
    wq = np.ascontiguousarray(np.asarray(inputs["Wq"], dtype=np.float32).astype(bf))
    wk = np.ascontiguousarray(np.asarray(inputs["Wk"], dtype=np.float32).astype(bf))
    wv = np.ascontiguousarray(np.asarray(inputs["Wv"], dtype=np.float32).astype(bf))

    if _CACHED_NC is None:
        _CACHED_NC = build_nc()
    nc = _CACHED_NC

    in_maps = [
        {"x": x[b], "context": ctxt[b], "Wq": wq, "Wk": wk, "Wv": wv}
        for b in range(B)
    ]
    res = run_bass_kernel_spmd(nc, in_maps, core_ids=list(range(B)))
    return np.stack(
        [res.results[b]["out"].astype(np.float32) for b in range(B)], axis=0
    )


# revision 27
# speedup vs baseline: 1.0063x; 1.0021x over previous
"""Cross-attention kernel for Trainium2 (Bass/Tile), 8-core data-parallel.

Reference computation (per batch element b):
    q = x @ Wq.T ; k = ctx @ Wk.T ; v = ctx @ Wv.T
    out = softmax((q @ k.T) * D**-0.5) @ v

Shapes: x [8, 2048, 1024], context [8, 2048, 1024], Wq/Wk/Wv [1024, 1024].

Strategy: pure data-parallel -- one batch element per NeuronCore, no
collectives.  All matmuls in bf16 with fp32 PSUM accumulation.

Since softmax((q k^T) * s) only needs q k^T = x (Wq^T Wk) ctx^T, we never
materialize q or k: W' = Wq^T Wk is computed from the *natural* weight
layouts (contraction over the out-feature axis, already on partitions),
then yT = W'^T x^T and dotsT = ctx^T-stationary x yT-moving.  This kills
the k projection and all Wq/Wk transposes.

PE-roofline design (v4):
  * Inputs are pre-cast to bf16 on the HOST: device DMA is half of
    fp32 and there are no on-chip casts at all.  (An XBAR
    transposed-load variant was measured slower -- 256B-granular HBM
    reads starve the PE under 8-core contention -- so transposes stay
    on the PE where they cost only ~67ns each back-to-back.)
  * 128x128 block transposes of Wv/ctx/x run on the PE in groups of 4
    sharing one PSUM bank; one strided DVE copy drains four blocks.
  * Softmax denominators come from N=1 matmuls (moving = ones [128,1])
    that reuse the attn^T stationary already loaded for the attn@v
    matmuls; sum_t attn[s,t] accumulates in a [128,1] PSUM column.
    ~25ns each instead of a second full pass of attn^T through the PE
    (~27us) plus column transposes.
  * Normalization (x 1/rowsum) runs on DVE so the Scalar engine does
    nothing but exp during the attention phase and a dots drain never
    queues behind a mul.
  * Softmax runs without max-subtraction (logits are O(5) for
    unit-normal inputs); exp comes straight out of PSUM on the Scalar
    engine with the 1/32 scale folded in; normalization is applied
    after the attn@v matmul since that matmul is linear in attn.
"""

from contextlib import ExitStack

import numpy as np

B = 8
S = 2048  # query length
T = 2048  # key/value length
D = 1024  # model dim
P = 128
SCALE = float(D) ** -0.5

N_ST = S // P  # 16 query tiles
N_TT = T // P  # 16 key tiles
N_DT = D // P  # 8 contraction chunks
NPROJ = D // 512  # 2 x 512-wide chunks for [., 1024] outputs
NSB = 4  # x is processed in 4 s-blocks of 512 rows for the yT projection
SH = S // 2  # attention processed in 2 query halves of 1024


def _emit_body(tc, x, ctxt, wq, wk, wv, out):
    import concourse.mybir as mybir
    from concourse.masks import make_identity

    fp32 = mybir.dt.float32
    bf16 = mybir.dt.bfloat16
    nc = tc.nc

    with ExitStack() as ctx:
        # Several pools share slots across phases via a common tag: wnp
        # hosts Wq/Wk naturals (phase A) then attn^T tiles (phase B);
        # xtbp hosts x^T blocks (phase A) then fp32 out staging (phase B).
        const = ctx.enter_context(tc.tile_pool(name="const", bufs=1))
        # 18 slots (2 spare) so dots-h1's first attn^T tiles can allocate
        # while attnv-h0 still holds all 16 h0 tiles -> no PE gap at the
        # half transition.
        wnp = ctx.enter_context(tc.tile_pool(name="wnp", bufs=18))
        castp = ctx.enter_context(tc.tile_pool(name="castp", bufs=8))
        wpp = ctx.enter_context(tc.tile_pool(name="wpp", bufs=8))
        wvtp = ctx.enter_context(tc.tile_pool(name="wvtp", bufs=2))
        xtbp = ctx.enter_context(tc.tile_pool(name="xtbp", bufs=4))
        ctxp = ctx.enter_context(tc.tile_pool(name="ctxp", bufs=2))
        ytp = ctx.enter_context(tc.tile_pool(name="ytp", bufs=8))
        vp = ctx.enter_context(tc.tile_pool(name="vp", bufs=16))
        smp = ctx.enter_context(tc.tile_pool(name="smp", bufs=2))

        ident_b = const.tile([P, P], bf16, name="ident_b")
        make_identity(nc, ident_b)
        ones_b = const.tile([P, 1], bf16, name="ones_b")
        nc.vector.memset(ones_b, 1.0)

        # Inputs are pre-cast to bf16 on the host, so a load is a single
        # bf16 DMA -- half the HBM traffic of fp32 and no cast op at all.
        # Loads alternate between the two HWDGE queues (SP and ACT) so two
        # row-tiles are always in flight.
        def load_bf(dram_rows, pool, tag, nm):
            """DMA one bf16 [128, D] row-tile."""
            bt = pool.tile([P, D], bf16, name=f"bf_{nm}", tag=tag)
            nc.sync.dma_start(out=bt, in_=dram_rows)
            return bt

        def prep_t(dram_rows, dst_for_group, psum_pool, nm):
            """Load one bf16 [128, D] row-tile and PE-transpose its 8
            128x128 blocks in 2 groups of 4 sharing one PSUM bank; one
            strided DVE copy per group scatters into dst_for_group(g) (an
            AP shaped [128, 4, 128])."""
            bt = load_bf(dram_rows, castp, "cast", nm)
            for g in range(2):
                ps = psum_pool.tile(
                    [P, 4 * P], bf16, name=f"tp_{nm}_{g}", tag="pt", bufs=3
                )
                for j in range(4):
                    nc.tensor.transpose(
                        ps[:, j * P : (j + 1) * P],
                        bt[:, (4 * g + j) * P : (4 * g + j + 1) * P],
                        ident_b,
                    )
                nc.vector.tensor_copy(
                    out=dst_for_group(g), in_=ps.rearrange("p (j c) -> p j c", j=4)
                )

        with tc.tile_pool(name="psumA", bufs=1, space="PSUM") as psA:
            # ---- Wv^T first (small), then ctx: each ctx row-tile's
            # transposes are chased immediately by that tile's v matmuls so
            # the PE stays fed at DMA pace ----
            wvg = [
                wvtp.tile([P, 4, D], bf16, name=f"wvg{g}", tag="wvg")
                for g in range(2)
            ]
            ctxg = [
                ctxp.tile([P, 4, T], bf16, name=f"ctxg{g}", tag="ctxg")
                for g in range(2)
            ]
            v = [vp.tile([P, D], bf16, name=f"v{t}", tag="v") for t in range(N_TT)]

            def prep_wv(rt):
                prep_t(
                    wv[rt * P : (rt + 1) * P, :],
                    lambda g, rt=rt: wvg[g][:, :, rt * P : (rt + 1) * P],
                    psA,
                    f"wv{rt}",
                )

            def prep_ctx(rt):
                prep_t(
                    ctxt[rt * P : (rt + 1) * P, :],
                    lambda g, rt=rt: ctxg[g][:, :, rt * P : (rt + 1) * P],
                    psA,
                    f"c{rt}",
                )

            # Wv columns 0-511 first so v[tt][ne=0] can start after just 4 Wv
            # tiles + one ctx tile; remaining Wv tiles stream in behind.
            for rt in range(4):
                prep_wv(rt)
            prep_ctx(0)
            for rt in range(4, N_DT):
                prep_wv(rt)
            for rt in range(N_TT):
                if rt > 0:
                    prep_ctx(rt)
                tt = rt  # v = ctx @ Wv^T, natural layout [t, e]
                for ne in range(NPROJ):
                    ps = psA.tile(
                        [P, 512], fp32, name=f"pv{tt}_{ne}", tag="proj", bufs=4
                    )
                    for d in range(N_DT):
                        nc.tensor.matmul(
                            ps,
                            ctxg[d // 4][:, d % 4, tt * P : (tt + 1) * P],
                            wvg[d // 4][:, d % 4, ne * 512 : (ne + 1) * 512],
                            start=(d == 0),
                            stop=(d == N_DT - 1),
                        )
                    nc.scalar.copy(out=v[tt][:, ne * 512 : (ne + 1) * 512], in_=ps)

            # ---- Wq/Wk in natural layout (bf16), then W' = Wq^T @ Wk ----
            wqn = [
                load_bf(wq[e * P : (e + 1) * P, :], wnp, "wn", f"wq{e}")
                for e in range(N_DT)
            ]
            wkn = [
                load_bf(wk[e * P : (e + 1) * P, :], wnp, "wn", f"wk{e}")
                for e in range(N_DT)
            ]
            wpb = [
                wpp.tile([P, D], bf16, name=f"wp{i}", tag="wp") for i in range(N_DT)
            ]
            for it in range(N_DT):
                for jn in range(NPROJ):
                    ps = psA.tile(
                        [P, 512], fp32, name=f"pw{it}_{jn}", tag="proj", bufs=4
                    )
                    for e in range(N_DT):
                        nc.tensor.matmul(
                            ps,
                            wqn[e][:, it * P : (it + 1) * P],
                            wkn[e][:, jn * 512 : (jn + 1) * 512],
                            start=(e == 0),
                            stop=(e == N_DT - 1),
                        )
                    nc.scalar.copy(out=wpb[it][:, jn * 512 : (jn + 1) * 512], in_=ps)

            # ---- yT = (x @ W')^T, streamed over 4 s-blocks of 512 ----
            yt = [
                ytp.tile([P, S], bf16, name=f"yt{j}", tag="yt") for j in range(N_DT)
            ]
            for sb in range(NSB):
                xtb = [
                    xtbp.tile([P, 4, 512], bf16, name=f"xtb{sb}_{g}", tag="xtb")
                    for g in range(2)
                ]
                for r in range(4):
                    rt = 4 * sb + r
                    prep_t(
                        x[rt * P : (rt + 1) * P, :],
                        lambda g, r=r: xtb[g][:, :, r * P : (r + 1) * P],
                        psA,
                        f"x{rt}",
                    )
                for jt in range(N_DT):
                    ps = psA.tile(
                        [P, 512], fp32, name=f"py{sb}_{jt}", tag="proj", bufs=4
                    )
                    for i in range(N_DT):
                        nc.tensor.matmul(
                            ps,
                            wpb[i][:, jt * P : (jt + 1) * P],
                            xtb[i // 4][:, i % 4, :],
                            start=(i == 0),
                            stop=(i == N_DT - 1),
                        )
                    nc.scalar.copy(
                        out=yt[jt][:, sb * 512 : (sb + 1) * 512], in_=ps
                    )

        # ---- attention ----
        # dots is produced TRANSPOSED: dotsT[t_tile, s] = sum_d ctxT[d, t] *
        # yT[d, s] (same operands as dots, roles swapped), so exp output IS
        # attn^T and the attn@v matmul needs no transposes at all.  S is
        # processed in 2 halves of 1024 so attn^T fits in the 16 SBUF slots
        # the W' inputs vacated.
        with tc.tile_pool(name="psumB", bufs=1, space="PSUM") as psB:
            for h in range(2):
                atT = []
                for tt in range(N_TT):
                    at = wnp.tile([P, SH], bf16, name=f"atT{h}_{tt}", tag="wn")
                    for ns in range(SH // 512):
                        ps = psB.tile(
                            [P, 512], fp32, name=f"pd{h}_{tt}_{ns}", tag="dots",
                            bufs=3,
                        )
                        for d in range(N_DT):
                            nc.tensor.matmul(
                                ps,
                                ctxg[d // 4][:, d % 4, tt * P : (tt + 1) * P],
                                yt[d][:, h * SH + ns * 512 : h * SH + (ns + 1) * 512],
                                start=(d == 0),
                                stop=(d == N_DT - 1),
                            )
                        nc.scalar.activation(
                            out=at[:, ns * 512 : (ns + 1) * 512],
                            in_=ps,
                            func=mybir.ActivationFunctionType.Exp,
                            scale=SCALE,
                        )
                    atT.append(at)

                for sl in range(N_ST // 2):
                    st = h * (N_ST // 2) + sl
                    out_sb = xtbp.tile([P, D], fp32, name=f"o{st}", tag="xtb")
                    ps0 = psB.tile(
                        [P, 512], fp32, name=f"pav{st}_0", tag="av", bufs=2
                    )
                    ps1 = psB.tile(
                        [P, 512], fp32, name=f"pav{st}_1", tag="av2", bufs=2
                    )
                    psr = psB.tile(
                        [P, 1], fp32, name=f"psr{st}", tag="rsum", bufs=1
                    )
                    for tt in range(N_TT):
                        a_sl = atT[tt][:, sl * P : (sl + 1) * P]
                        nc.tensor.matmul(
                            psr, a_sl, ones_b,
                            start=(tt == 0), stop=(tt == N_TT - 1),
                        )
                        nc.tensor.matmul(
                            ps0, a_sl, v[tt][:, 0:512],
                            start=(tt == 0), stop=(tt == N_TT - 1),
                        )
                        nc.tensor.matmul(
                            ps1, a_sl, v[tt][:, 512:1024],
                            start=(tt == 0), stop=(tt == N_TT - 1),
                        )
                    recip = smp.tile(
                        [P, 1], fp32, name=f"rc{st}", tag="recip", bufs=8
                    )
                    nc.vector.reciprocal(out=recip, in_=psr)
                    nc.vector.tensor_scalar_mul(
                        out_sb[:, 0:512], ps0, recip
                    )
                    nc.vector.tensor_scalar_mul(
                        out_sb[:, 512:1024], ps1, recip
                    )
                    nc.sync.dma_start(
                        out=out[st * P : (st + 1) * P, :], in_=out_sb
                    )


def build_nc():
    import concourse.mybir as mybir
    import concourse.tile as tile
    from concourse import bacc

    fp32 = mybir.dt.float32
    bf16 = mybir.dt.bfloat16
    nc = bacc.Bacc("TRN2", target_bir_lowering=False, debug=False)
    x = nc.dram_tensor("x", [S, D], bf16, kind="ExternalInput").ap()
    ctxt = nc.dram_tensor("context", [T, D], bf16, kind="ExternalInput").ap()
    wq = nc.dram_tensor("Wq", [D, D], bf16, kind="ExternalInput").ap()
    wk = nc.dram_tensor("Wk", [D, D], bf16, kind="ExternalInput").ap()
    wv = nc.dram_tensor("Wv", [D, D], bf16, kind="ExternalInput").ap()
    out = nc.dram_tensor("out", [S, D], fp32, kind="ExternalOutput").ap()
    with tile.TileContext(nc) as tc:
        _emit_body(tc, x, ctxt, wq, wk, wv, out)
    nc.compile()
    return nc


_CACHED_NC = None


def kernel(**inputs):
    global _CACHED_NC
    import ml_dtypes
    from concourse.bass_utils import run_bass_kernel_spmd

    bf = ml_dtypes.bfloat16
    x = np.ascontiguousarray(np.asarray(inputs["x"], dtype=np.float32).astype(bf))
    ctxt = np.ascontiguousarray(
        np.asarray(inputs["context"], dtype=np.float32).astype(bf)
    )
    wq = np.ascontiguousarray(np.asarray(inputs["Wq"], dtype=np.float32).astype(bf))
    wk = np.ascontiguousarray(np.asarray(inputs["Wk"], dtype=np.float32).astype(bf))
    wv = np.ascontiguousarray(np.asarray(inputs["Wv"], dtype=np.float32).astype(bf))

    if _CACHED_NC is None:
        _CACHED_NC = build_nc()
    nc = _CACHED_NC

    in_maps = [
        {"x": x[b], "context": ctxt[b], "Wq": wq, "Wk": wk, "Wv": wv}
        for b in range(B)
    ]
    res = run_bass_kernel_spmd(nc, in_maps, core_ids=list(range(B)))
    return np.stack(
        [res.results[b]["out"].astype(np.float32) for b in range(B)], axis=0
    )


# revision 31
# speedup vs baseline: 1.0065x; 1.0002x over previous
"""Cross-attention kernel for Trainium2 (Bass/Tile), 8-core data-parallel.

Reference computation (per batch element b):
    q = x @ Wq.T ; k = ctx @ Wk.T ; v = ctx @ Wv.T
    out = softmax((q @ k.T) * D**-0.5) @ v

Shapes: x [8, 2048, 1024], context [8, 2048, 1024], Wq/Wk/Wv [1024, 1024].

Strategy: pure data-parallel -- one batch element per NeuronCore, no
collectives.  All matmuls in bf16 with fp32 PSUM accumulation.

Since softmax((q k^T) * s) only needs q k^T = x (Wq^T Wk) ctx^T, we never
materialize q or k: W' = Wq^T Wk is computed from the *natural* weight
layouts (contraction over the out-feature axis, already on partitions),
then yT = W'^T x^T and dotsT = ctx^T-stationary x yT-moving.  This kills
the k projection and all Wq/Wk transposes.

PE-roofline design (v4):
  * Inputs are pre-cast to bf16 on the HOST: device DMA is half of
    fp32 and there are no on-chip casts at all.  (An XBAR
    transposed-load variant was measured slower -- 256B-granular HBM
    reads starve the PE under 8-core contention -- so transposes stay
    on the PE where they cost only ~67ns each back-to-back.)
  * 128x128 block transposes of Wv/ctx/x run on the PE in groups of 4
    sharing one PSUM bank; one strided DVE copy drains four blocks.
  * Softmax denominators come from N=1 matmuls (moving = ones [128,1])
    that reuse the attn^T stationary already loaded for the attn@v
    matmuls; sum_t attn[s,t] accumulates in a [128,1] PSUM column.
    ~25ns each instead of a second full pass of attn^T through the PE
    (~27us) plus column transposes.
  * Normalization (x 1/rowsum) runs on DVE so the Scalar engine does
    nothing but exp during the attention phase and a dots drain never
    queues behind a mul.
  * Softmax runs without max-subtraction (logits are O(5) for
    unit-normal inputs); exp comes straight out of PSUM on the Scalar
    engine with the 1/32 scale folded in; normalization is applied
    after the attn@v matmul since that matmul is linear in attn.
"""

from contextlib import ExitStack

import numpy as np

B = 8
S = 2048  # query length
T = 2048  # key/value length
D = 1024  # model dim
P = 128
SCALE = float(D) ** -0.5

N_ST = S // P  # 16 query tiles
N_TT = T // P  # 16 key tiles
N_DT = D // P  # 8 contraction chunks
NPROJ = D // 512  # 2 x 512-wide chunks for [., 1024] outputs
NSB = 4  # x is processed in 4 s-blocks of 512 rows for the yT projection
SH = S // 2  # attention processed in 2 query halves of 1024


def _emit_body(tc, x, ctxt, wq, wk, wv, out):
    import concourse.mybir as mybir
    from concourse.masks import make_identity

    fp32 = mybir.dt.float32
    bf16 = mybir.dt.bfloat16
    nc = tc.nc

    with ExitStack() as ctx:
        # Several pools share slots across phases via a common tag: wnp
        # hosts Wq/Wk naturals (phase A) then attn^T tiles (phase B);
        # xtbp hosts x^T blocks (phase A) then fp32 out staging (phase B).
        const = ctx.enter_context(tc.tile_pool(name="const", bufs=1))
        # 18 slots (2 spare) so dots-h1's first attn^T tiles can allocate
        # while attnv-h0 still holds all 16 h0 tiles -> no PE gap at the
        # half transition.
        wnp = ctx.enter_context(tc.tile_pool(name="wnp", bufs=18))
        castp = ctx.enter_context(tc.tile_pool(name="castp", bufs=8))
        wpp = ctx.enter_context(tc.tile_pool(name="wpp", bufs=8))
        wvtp = ctx.enter_context(tc.tile_pool(name="wvtp", bufs=2))
        xtbp = ctx.enter_context(tc.tile_pool(name="xtbp", bufs=4))
        ctxp = ctx.enter_context(tc.tile_pool(name="ctxp", bufs=2))
        ytp = ctx.enter_context(tc.tile_pool(name="ytp", bufs=8))
        vp = ctx.enter_context(tc.tile_pool(name="vp", bufs=16))
        smp = ctx.enter_context(tc.tile_pool(name="smp", bufs=2))

        ident_b = const.tile([P, P], bf16, name="ident_b")
        make_identity(nc, ident_b)
        ones_b = const.tile([P, 1], bf16, name="ones_b")
        nc.vector.memset(ones_b, 1.0)

        # HAM warm-up: the PE clock-gate starts at 4/8 (1.2 GHz) and only
        # reaches 8/8 after ~3.4us of sustained activity.  The first input
        # DMA takes ~2.5us during which the PE would idle cold; burn that
        # window on identity transposes instead (I^T @ I = I, all reading
        # ident_b so they run back-to-back on the PE; the WAW copy chain
        # into ident_w keeps them DCE-live because every real transpose
        # consumes ident_w as its identity operand).
        ident_w = const.tile([P, P], bf16, name="ident_w")
        with tc.tile_pool(name="warm", bufs=2, space="PSUM") as wps:
            for w in range(10):
                wp = wps.tile([P, P], bf16, name=f"warm{w}", tag="warm", bufs=2)
                nc.tensor.transpose(wp, ident_b, ident_b)
                nc.vector.tensor_copy(out=ident_w, in_=wp)

        # Inputs are pre-cast to bf16 on the host, so a load is a single
        # bf16 DMA -- half the HBM traffic of fp32 and no cast op at all.
        # Loads alternate between the two HWDGE queues (SP and ACT) so two
        # row-tiles are always in flight.
        def load_bf(dram_rows, pool, tag, nm):
            """DMA one bf16 [128, D] row-tile."""
            bt = pool.tile([P, D], bf16, name=f"bf_{nm}", tag=tag)
            nc.sync.dma_start(out=bt, in_=dram_rows)
            return bt

        def prep_t(dram_rows, dst_for_group, psum_pool, nm):
            """Load one bf16 [128, D] row-tile and PE-transpose its 8
            128x128 blocks in 2 groups of 4 sharing one PSUM bank; one
            strided DVE copy per group scatters into dst_for_group(g) (an
            AP shaped [128, 4, 128])."""
            bt = load_bf(dram_rows, castp, "cast", nm)
            for g in range(2):
                ps = psum_pool.tile(
                    [P, 4 * P], bf16, name=f"tp_{nm}_{g}", tag="pt", bufs=3
                )
                for j in range(4):
                    nc.tensor.transpose(
                        ps[:, j * P : (j + 1) * P],
                        bt[:, (4 * g + j) * P : (4 * g + j + 1) * P],
                        ident_w,
                    )
                nc.vector.tensor_copy(
                    out=dst_for_group(g), in_=ps.rearrange("p (j c) -> p j c", j=4)
                )

        with tc.tile_pool(name="psumA", bufs=1, space="PSUM") as psA:
            # ---- Wv^T first (small), then ctx: each ctx row-tile's
            # transposes are chased immediately by that tile's v matmuls so
            # the PE stays fed at DMA pace ----
            wvg = [
                wvtp.tile([P, 4, D], bf16, name=f"wvg{g}", tag="wvg")
                for g in range(2)
            ]
            ctxg = [
                ctxp.tile([P, 4, T], bf16, name=f"ctxg{g}", tag="ctxg")
                for g in range(2)
            ]
            v = [vp.tile([P, D], bf16, name=f"v{t}", tag="v") for t in range(N_TT)]

            def prep_wv(rt):
                prep_t(
                    wv[rt * P : (rt + 1) * P, :],
                    lambda g, rt=rt: wvg[g][:, :, rt * P : (rt + 1) * P],
                    psA,
                    f"wv{rt}",
                )

            def prep_ctx(rt):
                prep_t(
                    ctxt[rt * P : (rt + 1) * P, :],
                    lambda g, rt=rt: ctxg[g][:, :, rt * P : (rt + 1) * P],
                    psA,
                    f"c{rt}",
                )

            # Wv columns 0-511 first so v[tt][ne=0] can start after just 4 Wv
            # tiles + one ctx tile; remaining Wv tiles stream in behind.
            for rt in range(4):
                prep_wv(rt)
            prep_ctx(0)
            for rt in range(4, N_DT):
                prep_wv(rt)
            for rt in range(N_TT):
                if rt > 0:
                    prep_ctx(rt)
                tt = rt  # v = ctx @ Wv^T, natural layout [t, e]
                for ne in range(NPROJ):
                    ps = psA.tile(
                        [P, 512], fp32, name=f"pv{tt}_{ne}", tag="proj", bufs=4
                    )
                    for d in range(N_DT):
                        nc.tensor.matmul(
                            ps,
                            ctxg[d // 4][:, d % 4, tt * P : (tt + 1) * P],
                            wvg[d // 4][:, d % 4, ne * 512 : (ne + 1) * 512],
                            start=(d == 0),
                            stop=(d == N_DT - 1),
                        )
                    nc.scalar.copy(out=v[tt][:, ne * 512 : (ne + 1) * 512], in_=ps)

            # ---- Wq/Wk in natural layout (bf16), then W' = Wq^T @ Wk ----
            wqn = [
                load_bf(wq[e * P : (e + 1) * P, :], wnp, "wn", f"wq{e}")
                for e in range(N_DT)
            ]
            wkn = [
                load_bf(wk[e * P : (e + 1) * P, :], wnp, "wn", f"wk{e}")
                for e in range(N_DT)
            ]
            wpb = [
                wpp.tile([P, D], bf16, name=f"wp{i}", tag="wp") for i in range(N_DT)
            ]
            for it in range(N_DT):
                for jn in range(NPROJ):
                    ps = psA.tile(
                        [P, 512], fp32, name=f"pw{it}_{jn}", tag="proj", bufs=4
                    )
                    for e in range(N_DT):
                        nc.tensor.matmul(
                            ps,
                            wqn[e][:, it * P : (it + 1) * P],
                            wkn[e][:, jn * 512 : (jn + 1) * 512],
                            start=(e == 0),
                            stop=(e == N_DT - 1),
                        )
                    nc.scalar.copy(out=wpb[it][:, jn * 512 : (jn + 1) * 512], in_=ps)

            # ---- yT = (x @ W')^T, streamed over 4 s-blocks of 512 ----
            yt = [
                ytp.tile([P, S], bf16, name=f"yt{j}", tag="yt") for j in range(N_DT)
            ]
            for sb in range(NSB):
                xtb = [
                    xtbp.tile([P, 4, 512], bf16, name=f"xtb{sb}_{g}", tag="xtb")
                    for g in range(2)
                ]
                for r in range(4):
                    rt = 4 * sb + r
                    prep_t(
                        x[rt * P : (rt + 1) * P, :],
                        lambda g, r=r: xtb[g][:, :, r * P : (r + 1) * P],
                        psA,
                        f"x{rt}",
                    )
                for jt in range(N_DT):
                    ps = psA.tile(
                        [P, 512], fp32, name=f"py{sb}_{jt}", tag="proj", bufs=4
                    )
                    for i in range(N_DT):
                        nc.tensor.matmul(
                            ps,
                            wpb[i][:, jt * P : (jt + 1) * P],
                            xtb[i // 4][:, i % 4, :],
                            start=(i == 0),
                            stop=(i == N_DT - 1),
                        )
                    nc.scalar.copy(
                        out=yt[jt][:, sb * 512 : (sb + 1) * 512], in_=ps
                    )

        # ---- attention ----
        # dots is produced TRANSPOSED: dotsT[t_tile, s] = sum_d ctxT[d, t] *
        # yT[d, s] (same operands as dots, roles swapped), so exp output IS
        # attn^T and the attn@v matmul needs no transposes at all.  S is
        # processed in 2 halves of 1024 so attn^T fits in the 16 SBUF slots
        # the W' inputs vacated.
        with tc.tile_pool(name="psumB", bufs=1, space="PSUM") as psB:
            for h in range(2):
                atT = []
                for tt in range(N_TT):
                    at = wnp.tile([P, SH], bf16, name=f"atT{h}_{tt}", tag="wn")
                    for ns in range(SH // 512):
                        ps = psB.tile(
                            [P, 512], fp32, name=f"pd{h}_{tt}_{ns}", tag="dots",
                            bufs=3,
                        )
                        for d in range(N_DT):
                            nc.tensor.matmul(
                                ps,
                                ctxg[d // 4][:, d % 4, tt * P : (tt + 1) * P],
                                yt[d][:, h * SH + ns * 512 : h * SH + (ns + 1) * 512],
                                start=(d == 0),
                                stop=(d == N_DT - 1),
                            )
                        nc.scalar.activation(
                            out=at[:, ns * 512 : (ns + 1) * 512],
                            in_=ps,
                            func=mybir.ActivationFunctionType.Exp,
                            scale=SCALE,
                        )
                    atT.append(at)

                for sl in range(N_ST // 2):
                    st = h * (N_ST // 2) + sl
                    out_sb = xtbp.tile([P, D], fp32, name=f"o{st}", tag="xtb")
                    ps0 = psB.tile(
                        [P, 512], fp32, name=f"pav{st}_0", tag="av", bufs=2
                    )
                    ps1 = psB.tile(
                        [P, 512], fp32, name=f"pav{st}_1", tag="av2", bufs=2
                    )
                    psr = psB.tile(
                        [P, 1], fp32, name=f"psr{st}", tag="rsum", bufs=1
                    )
                    for tt in range(N_TT):
                        a_sl = atT[tt][:, sl * P : (sl + 1) * P]
                        nc.tensor.matmul(
                            psr, a_sl, ones_b,
                            start=(tt == 0), stop=(tt == N_TT - 1),
                        )
                        nc.tensor.matmul(
                            ps0, a_sl, v[tt][:, 0:512],
                            start=(tt == 0), stop=(tt == N_TT - 1),
                        )
                        nc.tensor.matmul(
                            ps1, a_sl, v[tt][:, 512:1024],
                            start=(tt == 0), stop=(tt == N_TT - 1),
                        )
                    recip = smp.tile(
                        [P, 1], fp32, name=f"rc{st}", tag="recip", bufs=8
                    )
                    nc.vector.reciprocal(out=recip, in_=psr)
                    # two half-width out DMAs: the first overlaps the second
                    # mul, halving the post-last-matmul drain chain
                    nc.vector.tensor_scalar_mul(
                        out_sb[:, 0:512], ps0, recip
                    )
                    nc.sync.dma_start(
                        out=out[st * P : (st + 1) * P, 0:512],
                        in_=out_sb[:, 0:512],
                    )
                    nc.vector.tensor_scalar_mul(
                        out_sb[:, 512:1024], ps1, recip
                    )
                    nc.sync.dma_start(
                        out=out[st * P : (st + 1) * P, 512:1024],
                        in_=out_sb[:, 512:1024],
                    )


def build_nc():
    import concourse.mybir as mybir
    import concourse.tile as tile
    from concourse import bacc

    fp32 = mybir.dt.float32
    bf16 = mybir.dt.bfloat16
    nc = bacc.Bacc("TRN2", target_bir_lowering=False, debug=False)
    x = nc.dram_tensor("x", [S, D], bf16, kind="ExternalInput").ap()
    ctxt = nc.dram_tensor("context", [T, D], bf16, kind="ExternalInput").ap()
    wq = nc.dram_tensor("Wq", [D, D], bf16, kind="ExternalInput").ap()
    wk = nc.dram_tensor("Wk", [D, D], bf16, kind="ExternalInput").ap()
    wv = nc.dram_tensor("Wv", [D, D], bf16, kind="ExternalInput").ap()
    out = nc.dram_tensor("out", [S, D], fp32, kind="ExternalOutput").ap()
    with tile.TileContext(nc) as tc:
        _emit_body(tc, x, ctxt, wq, wk, wv, out)
    nc.compile()
    return nc


_CACHED_NC = None


def kernel(**inputs):
    global _CACHED_NC
    import ml_dtypes
    from concourse.bass_utils import run_bass_kernel_spmd

    bf = ml_dtypes.bfloat16
    x = np.ascontiguousarray(np.asarray(inputs["x"], dtype=np.float32).astype(bf))
    ctxt = np.ascontiguousarray(
        np.asarray(inputs["context"], dtype=np.float32).astype(bf)
    )
    wq = np.ascontiguousarray(np.asarray(inputs["Wq"], dtype=np.float32).astype(bf))
    wk = np.ascontiguousarray(np.asarray(inputs["Wk"], dtype=np.float32).astype(bf))
    wv = np.ascontiguousarray(np.asarray(inputs["Wv"], dtype=np.float32).astype(bf))

    if _CACHED_NC is None:
        _CACHED_NC = build_nc()
    nc = _CACHED_NC

    in_maps = [
        {"x": x[b], "context": ctxt[b], "Wq": wq, "Wk": wk, "Wv": wv}
        for b in range(B)
    ]
    res = run_bass_kernel_spmd(nc, in_maps, core_ids=list(range(B)))
    return np.stack(
        [res.results[b]["out"].astype(np.float32) for b in range(B)], axis=0
    )
